# revision 2
# baseline (speedup 1.0000x reference)
"""MoE gate (router) kernel for Trainium2 — fp16 main + fp8-DoubleRow cross.

Computes, for hidden_states [T, H] and gate weight [E, H]:
    logits = hidden_states @ weight.T          # [T, E]
    probs  = softmax(logits, axis=-1)
    topk_weight, topk_idx = top_k(probs, 8)    # normalized over the top-8
    row_idx = arange(T*8).reshape(8, T).T

Strategy (8 NeuronCores, data parallel over tokens):
  - Accuracy from a 2-matmul/k-tile split (vs the 3-matmul fp16 hi/lo
    baseline): per k-tile,
      main : psum_m += hshi16 . whi16          (fp16, N=256, exact)
      cross: psum_c += hshi8.wlo8 + hslo8.whi8 (ONE fp8 DoubleRow matmul:
             DR sums lhsT[:,i,:].T @ rhs[:,i,:] over the pair dim i)
    with scale-matched fp8 operands (all in e4m3 normal range):
      hshi8 = fp8(hs)              wlo8  = fp8((w - whi16) * 2^20)
      hslo8 = fp8((hs-hshi)*2^13)  whi8s = fp8(w * 2^7)
    so both pair products carry 2^20:
      logits = psum_m + 2^-20 * psum_c
    Residual-operand errors are ~2^-18-relative -> ~2e-6 logit sigma,
    i.e. top-k exact up to a handful of 1-ulp ties (well under the
    rel-err budget).  The DR matmul covers contraction 256 in ~128
    cycles, so cross costs ~half a main matmul instead of two.
  - hs packed host-side into k-half-contiguous tiles (fp16 plane +
    pre-interleaved fp8 pair plane); each 128-token tile loads as four
    fully contiguous >=7 KiB-per-partition DMAs, keeping PE idle gaps
    under the ~3.4 us HAM re-throttle window.
  - Weights (whi16 + fp8 pair plane, ~7.3 MB) stream in chunks
    interleaved with the first token tiles; ALL weight DMAs are emitted
    before any matmul (Tile dep tracker orders only read-after-write
    for earlier writes in program order).
  - DVE max/max_index give top-8 values + indices per token in one
    instruction each; softmax over 256 experts + top-k renormalization
    reduces to a softmax over just the top-8 logits.
  - Output staging accumulates per-tile results in SBUF and leaves as
    four large-descriptor DMAs, the first three overlapping later
    tiles' compute.
"""

import numpy as np

TOP_K = 8
NUM_EXPERTS = 256
HIDDEN = 7168
NUM_TOKENS = 16384
N_CORES = 8
T_LOC = NUM_TOKENS // N_CORES

CROSS_SCALE = float(2.0**20)   # wlo8 carries 2^20; hslo8*whi8s = 2^13 * 2^7
HSLO_SCALE = float(2.0**13)
WHI8_SCALE = float(2.0**7)

_NC_CACHE = {}


def build_gate_nc(t_loc=T_LOC, h=HIDDEN, e=NUM_EXPERTS, repeat=1, serialize=False):
    import concourse.mybir as mybir
    import concourse.tile as tile
    from concourse import bacc
    from concourse.tile import add_dep_helper

    f32 = mybir.dt.float32
    fp16 = mybir.dt.float16
    fp8 = mybir.dt.float8e4
    P = 128
    KT = h // P          # k-tiles along hidden dim (56)
    KH = KT // 2         # k-half width (28)
    TS = t_loc // P      # 128-token subtiles per core (16)
    W_CHUNK_LENS = [2, 6, 8, 8, 8, 8, 8, 8]
    assert sum(W_CHUNK_LENS) == KT
    W_CHUNK_STARTS = [sum(W_CHUNK_LENS[:i]) for i in range(len(W_CHUNK_LENS))]

    def k_to_chunk(k):
        for ci in range(len(W_CHUNK_LENS) - 1, -1, -1):
            if k >= W_CHUNK_STARTS[ci]:
                return ci, k - W_CHUNK_STARTS[ci]
        raise AssertionError

    nc = bacc.Bacc("TRN2", target_bir_lowering=False)
    # fp16 hs plane: [p][ts][kh][t][ko2], k = p*KT + kh*KH + ko2
    hs16_pack = nc.dram_tensor(
        "hs16_pack", [P, TS, 2, P, KH], fp16, kind="ExternalInput"
    )
    # fp8 hs pair plane: [p][ts][kh][ko2][pair][t], pair 0 = hshi8, 1 = hslo8
    hs8_pack = nc.dram_tensor(
        "hs8_pack", [P, TS, 2, KH, 2, P], fp8, kind="ExternalInput"
    )
    # w fp16 plane [p][ko][e] and fp8 pair plane [p][ko][pair][e]
    w16_pack = nc.dram_tensor("w16_pack", [P, KT, e], fp16, kind="ExternalInput")
    w8_pack = nc.dram_tensor("w8_pack", [P, KT, 2, e], fp8, kind="ExternalInput")
    idx_out = nc.dram_tensor(
        "topk_idx", [t_loc, TOP_K], mybir.dt.int32, kind="ExternalOutput"
    )
    w_out = nc.dram_tensor("topk_w", [t_loc, TOP_K], f32, kind="ExternalOutput")

    with tile.TileContext(nc) as tc:
        with (
            tc.tile_pool(name="wpool", bufs=1) as wpool,
            tc.tile_pool(name="hpool", bufs=4) as hpool,
            tc.tile_pool(name="lpool", bufs=3) as lpool,
            tc.tile_pool(name="spool", bufs=4) as spool,
            tc.tile_pool(name="psum", bufs=4, space="PSUM") as psum_pool,
        ):
            if repeat > 0:
                stage_idx = wpool.tile([P, TS, TOP_K], mybir.dt.int32, tag="sidx")
                stage_wv = wpool.tile([P, TS, TOP_K], f32, tag="swv")
            w16_chunks, w8_chunks = [], []
            for wc_i, clen in enumerate(W_CHUNK_LENS):
                c16 = wpool.tile([P, clen, e], fp16, tag=f"w16_{wc_i}", name=f"w16_{wc_i}")
                c8 = wpool.tile([P, clen, 2, e], fp8, tag=f"w8_{wc_i}", name=f"w8_{wc_i}")
                w16_chunks.append(c16)
                w8_chunks.append(c8)

            def issue_w(wc_i):
                s0, l0 = W_CHUNK_STARTS[wc_i], W_CHUNK_LENS[wc_i]
                nc.sync.dma_start(w16_chunks[wc_i], w16_pack[:, s0 : s0 + l0, :])
                nc.sync.dma_start(w8_chunks[wc_i], w8_pack[:, s0 : s0 + l0, :, :])

            def issue_hs16(rep, ts_i):
                h16 = hpool.tile(
                    [P, 2, P, KH], fp16, tag="hs16", name=f"hs16_{rep}_{ts_i}"
                )
                ds = [
                    nc.sync.dma_start(h16[:, kh], hs16_pack[:, ts_i, kh])
                    for kh in range(2)
                ]
                return h16, ds

            def issue_hs8(rep, ts_i):
                h8 = hpool.tile(
                    [P, 2, KH, 2, P], fp8, tag="hs8", name=f"hs8_{rep}_{ts_i}"
                )
                ds = [
                    nc.sync.dma_start(h8[:, kh], hs8_pack[:, ts_i, kh])
                    for kh in range(2)
                ]
                return h8, ds

            def issue_hs(rep, ts_i):
                # four DMAs per 128-token tile (fp16/fp8 x k-halves), each
                # fully contiguous >=7,168 B per partition
                h16, d16 = issue_hs16(rep, ts_i)
                h8, d8 = issue_hs8(rep, ts_i)
                return h16, h8, d16 + d8

            N_PRE = min(4, TS) if repeat > 0 else 0
            pre_hs = {}
            dma_done = {}
            _clock = [0.0]

            def _track(kind, key, nbytes):
                _clock[0] += nbytes / 360.0
                dma_done[(kind, key)] = _clock[0] + 1500.0

            H16_BYTES = P * P * KH * 2
            H8_BYTES = P * KH * 2 * P

            def w_bytes(ci):
                return P * W_CHUNK_LENS[ci] * e * (2 + 2)

            def track_hs(ts_i):
                _track("h16_0", ts_i, H16_BYTES)
                _track("h8_0", ts_i, H8_BYTES)
                _track("h16_1", ts_i, H16_BYTES)
                _track("h8_1", ts_i, H8_BYTES)

            issue_w(0); _track("w", 0, w_bytes(0))
            if N_PRE > 0:
                pre_hs[0] = issue_hs(0, 0); track_hs(0)
            issue_w(1); _track("w", 1, w_bytes(1))
            if N_PRE > 1:
                pre_hs[1] = issue_hs(0, 1); track_hs(1)
            issue_w(2); _track("w", 2, w_bytes(2))
            issue_w(3); _track("w", 3, w_bytes(3))
            if N_PRE > 2:
                pre_hs[2] = issue_hs(0, 2); track_hs(2)
            issue_w(4); _track("w", 4, w_bytes(4))
            issue_w(5); _track("w", 5, w_bytes(5))
            if N_PRE > 3:
                pre_hs[3] = issue_hs(0, 3); track_hs(3)
            issue_w(6); _track("w", 6, w_bytes(6))
            issue_w(7); _track("w", 7, w_bytes(7))

            def mm_main(ptm, h16, k, start, stop):
                ci, ki = k_to_chunk(k)
                nc.tensor.matmul(
                    ptm, h16[:, k // KH, :, k % KH], w16_chunks[ci][:, ki, :],
                    start=start, stop=stop,
                )

            def mm_cross(ptc, h8, k, start, stop):
                ci, ki = k_to_chunk(k)
                nc.tensor.matmul(
                    ptc, h8[:, k // KH, k % KH], w8_chunks[ci][:, ki],
                    start=start, stop=stop,
                    perf_mode=mybir.MatmulPerfMode.DoubleRow,
                )

            # fill-phase matmuls for prefetched tiles, in data-ready order
            pre_pt = {}
            if N_PRE:
                sched = []
                for ts_i in range(N_PRE):
                    ptm = psum_pool.tile([P, e], f32, tag="ptm", name=f"ptm_pre{ts_i}")
                    ptc = psum_pool.tile([P, e], f32, tag="ptc", name=f"ptc_pre{ts_i}")
                    pre_pt[ts_i] = (ptm, ptc)
                    for k in range(KT):
                        ci, _ = k_to_chunk(k)
                        kh = k // KH
                        rdy_m = max(dma_done[("w", ci)], dma_done[(f"h16_{kh}", ts_i)])
                        rdy_c = max(dma_done[("w", ci)], dma_done[(f"h8_{kh}", ts_i)])
                        # kind-major within each readiness step keeps same-mode
                        # matmuls grouped (mode switches stall the PE weight
                        # pipeline)
                        sched.append((rdy_m, 0, ts_i, k))
                        sched.append((rdy_c, 1, ts_i, k))
                sched.sort()
                started = {}
                last_of = {}
                for i, (_, kind, ts_i, k) in enumerate(sched):
                    last_of[(ts_i, kind)] = i
                for i, (_, kind, ts_i, k) in enumerate(sched):
                    h16, h8, _ = pre_hs[ts_i]
                    ptm, ptc = pre_pt[ts_i]
                    key = (ts_i, kind)
                    st = key not in started
                    started[key] = True
                    sp = i == last_of[key]
                    if kind == 0:
                        mm_main(ptm, h16, k, st, sp)
                    else:
                        mm_cross(ptc, h8, k, st, sp)

            def epilogue(rep, ts_i, ptm, ptc):
                nonlocal prev_tail
                # logits = psum_main + 2^-20 * psum_cross
                lg = lpool.tile([P, e], f32, tag="lg")
                nc.vector.tensor_scalar_mul(lg, ptc, 1.0 / CROSS_SCALE)
                nc.vector.tensor_add(lg, lg, ptm)
                mx = spool.tile([P, TOP_K], f32, tag="mx")
                nc.vector.max(out=mx, in_=lg)
                idx_u = spool.tile([P, TOP_K], mybir.dt.uint32, tag="idxu")
                nc.vector.max_index(idx_u, mx, lg)
                nc.vector.tensor_copy(stage_idx[:, ts_i, :], idx_u)
                nm = spool.tile([P, 1], f32, tag="nm")
                nc.vector.tensor_scalar_mul(nm, mx[:, 0:1], -1.0)
                ev = spool.tile([P, TOP_K], f32, tag="ev")
                sm = spool.tile([P, 1], f32, tag="sm")
                nc.scalar.activation(
                    ev,
                    mx,
                    mybir.ActivationFunctionType.Exp,
                    bias=nm,
                    scale=1.0,
                    accum_out=sm,
                )
                rc = spool.tile([P, 1], f32, tag="rc")
                nc.vector.reciprocal(rc, sm)
                tail = nc.vector.tensor_scalar_mul(stage_wv[:, ts_i, :], ev, rc)
                if serialize and ts_i == TS - 1:
                    prev_tail = tail
                if rep == repeat - 1 and (ts_i + 1) % (TS // 4) == 0 and ts_i != TS - 1:
                    q0 = (ts_i + 1 - TS // 4) * P
                    q1 = (ts_i + 1) * P
                    nc.sync.dma_start(
                        idx_out[q0:q1, :].rearrange("(ts p) k -> p ts k", p=P),
                        stage_idx[:, ts_i + 1 - TS // 4 : ts_i + 1, :],
                    )
                    nc.sync.dma_start(
                        w_out[q0:q1, :].rearrange("(ts p) k -> p ts k", p=P),
                        stage_wv[:, ts_i + 1 - TS // 4 : ts_i + 1, :],
                    )

            prev_tail = None
            for rep in range(repeat):
                ts_i = 0
                while ts_i < TS:
                    if rep == 0 and ts_i < N_PRE:
                        h16, h8, ds = pre_hs[ts_i]
                        ptm, ptc = pre_pt[ts_i]
                        epilogue(rep, ts_i, ptm, ptc)
                        ts_i += 1
                        continue
                    # 2-tile groups: A(t0) A(t1) B(t0) B(t1) halves the
                    # fp16<->DoubleRow mode transitions (each stalls the PE
                    # weight pipeline ~0.2 us); h16 DMAs lead h8 so the A
                    # phases are fed first
                    group = [ts_i] + ([ts_i + 1] if ts_i + 1 < TS else [])
                    datas = []
                    for t in group:
                        h16, d16 = issue_hs16(rep, t)
                        datas.append([t, h16, None, None, None, d16])
                    for di, t in enumerate(group):
                        h8, d8 = issue_hs8(rep, t)
                        datas[di][2] = h8
                        datas[di][5] = datas[di][5] + d8
                    if serialize and prev_tail is not None:
                        for row in datas:
                            for d in row[5]:
                                add_dep_helper(prev_tail.ins, d.ins, reason="ser-rep")
                    for row in datas:
                        row[3] = psum_pool.tile(
                            [P, e], f32, tag="ptm", name=f"ptm{rep}_{row[0]}"
                        )
                        row[4] = psum_pool.tile(
                            [P, e], f32, tag="ptc", name=f"ptc{rep}_{row[0]}"
                        )
                    for t, h16, h8, ptm, ptc, _ in datas:
                        for k in range(KT):
                            mm_main(ptm, h16, k, k == 0, k == KT - 1)
                    for t, h16, h8, ptm, ptc, _ in datas:
                        for k in range(KT):
                            mm_cross(ptc, h8, k, k == 0, k == KT - 1)
                    for t, h16, h8, ptm, ptc, _ in datas:
                        epilogue(rep, t, ptm, ptc)
                    ts_i += len(group)
            if repeat > 0:
                q0 = (TS - TS // 4) * P
                nc.sync.dma_start(
                    idx_out[q0:, :].rearrange("(ts p) k -> p ts k", p=P),
                    stage_idx[:, TS - TS // 4 :, :],
                )
                nc.sync.dma_start(
                    w_out[q0:, :].rearrange("(ts p) k -> p ts k", p=P),
                    stage_wv[:, TS - TS // 4 :, :],
                )
    nc.compile()
    return nc


def _get_nc():
    key = (T_LOC, HIDDEN, NUM_EXPERTS)
    if key not in _NC_CACHE:
        _NC_CACHE[key] = build_gate_nc(*key)
    return _NC_CACHE[key]


def _prep_inputs(hs, w):
    from concourse import mybir

    f8 = mybir.dt.np(mybir.dt.float8e4)
    P = 128
    KT = HIDDEN // P
    KH = KT // 2
    TS = T_LOC // P
    E = NUM_EXPERTS

    whi = w.astype(np.float16)
    wres = (w.astype(np.float64) - whi.astype(np.float64)).astype(np.float32)
    wlo8 = (wres * np.float32(CROSS_SCALE)).astype(f8)
    whi8s = (w * np.float32(WHI8_SCALE)).astype(f8)
    # [E, H] -> [H, E] -> [p, ko, e]
    w16_pack = np.ascontiguousarray(whi.T).reshape(P, KT, E)
    w8_pack = np.empty((P, KT, 2, E), dtype=f8)
    w8_pack[:, :, 0, :] = np.ascontiguousarray(wlo8.T).reshape(P, KT, E)
    w8_pack[:, :, 1, :] = np.ascontiguousarray(whi8s.T).reshape(P, KT, E)

    def pack_core(c):
        hs_c = hs[c * T_LOC : (c + 1) * T_LOC]  # [T_LOC, H] f32
        hshi = hs_c.astype(np.float16)
        hsres = (hs_c.astype(np.float64) - hshi.astype(np.float64)).astype(
            np.float32
        )
        hshi8 = hs_c.astype(f8)
        hslo8 = (hsres * np.float32(HSLO_SCALE)).astype(f8)
        # fp16 plane: [ts, t, p, kh, ko2] -> [p, ts, kh, t, ko2]
        p16 = np.ascontiguousarray(
            hshi.reshape(TS, P, P, 2, KH).transpose(2, 0, 3, 1, 4)
        )
        # fp8 pair plane: [ts, t, p, kh, ko2] -> [p, ts, kh, ko2, pair, t]
        p8 = np.empty((P, TS, 2, KH, 2, P), dtype=f8)
        p8[:, :, :, :, 0, :] = hshi8.reshape(TS, P, P, 2, KH).transpose(
            2, 0, 3, 4, 1
        )
        p8[:, :, :, :, 1, :] = hslo8.reshape(TS, P, P, 2, KH).transpose(
            2, 0, 3, 4, 1
        )
        return {
            "hs16_pack": p16,
            "hs8_pack": p8,
            "w16_pack": w16_pack,
            "w8_pack": w8_pack,
        }

    from concurrent.futures import ThreadPoolExecutor

    with ThreadPoolExecutor(max_workers=N_CORES) as ex:
        in_maps = list(ex.map(pack_core, range(N_CORES)))
    return in_maps


_FN_CACHE = {}


def _make_runner(nc):
    """Compile a reusable 8-core PJRT callable."""
    import jax
    import concourse.mybir as mybir
    from concourse import bass2jax
    from jax.sharding import Mesh, NamedSharding, PartitionSpec
    from jax.experimental.shard_map import shard_map

    bass2jax.install_neuronx_cc_hook()
    partition_name = nc.partition_id_tensor.name if nc.partition_id_tensor else None
    in_names, out_names, out_avals, zero_shapes = [], [], [], []
    for alloc in nc.m.functions[0].allocations:
        if not isinstance(alloc, mybir.MemoryLocationSet):
            continue
        name = alloc.memorylocations[0].name
        if alloc.kind == "ExternalInput":
            if name != partition_name:
                in_names.append(name)
        elif alloc.kind == "ExternalOutput":
            shape = tuple(alloc.tensor_shape)
            dtype = mybir.dt.np(alloc.dtype)
            out_names.append(name)
            out_avals.append(jax.core.ShapedArray(shape, dtype))
            zero_shapes.append((shape, dtype))
    n_params = len(in_names)
    n_outs = len(out_avals)
    all_in_names = list(in_names) + list(out_names)
    if partition_name is not None:
        all_in_names.append(partition_name)

    def _body(*args):
        operands = list(args)
        if partition_name is not None:
            operands.append(bass2jax.partition_id_tensor())
        outs = bass2jax._bass_exec_p.bind(
            *operands,
            out_avals=tuple(out_avals),
            in_names=tuple(all_in_names),
            out_names=tuple(out_names),
            lowering_input_output_aliases=(),
            sim_require_finite=True,
            sim_require_nnan=True,
            nc=nc,
        )
        return tuple(outs)

    devices = jax.devices()[:N_CORES]
    mesh = Mesh(np.asarray(devices), ("core",))
    in_specs = (PartitionSpec("core"),) * (n_params + n_outs)
    out_specs = (PartitionSpec("core"),) * len(out_names)
    donate = tuple(range(n_params, n_params + n_outs))
    fn = jax.jit(
        shard_map(
            _body, mesh=mesh, in_specs=in_specs, out_specs=out_specs, check_rep=False
        ),
        donate_argnums=donate,
        keep_unused=True,
    )
    sharding = NamedSharding(mesh, PartitionSpec("core"))

    def run(in_maps):
        concat_in = [
            np.concatenate(
                [np.asarray(in_maps[c][nm]) for c in range(N_CORES)], axis=0
            )
            for nm in in_names
        ]
        zeros = [
            np.zeros((N_CORES * s[0], *s[1:]), dt) for s, dt in zero_shapes
        ]
        dev_in = [jax.device_put(x, sharding) for x in concat_in]
        out_arrs = fn(*dev_in, *zeros)
        return [
            {
                nm: np.asarray(out_arrs[i]).reshape(
                    N_CORES, *out_avals[i].shape
                )[c]
                for i, nm in enumerate(out_names)
            }
            for c in range(N_CORES)
        ]

    return run


def kernel(hidden_states, weight):
    hs = np.asarray(hidden_states, dtype=np.float32)
    w = np.asarray(weight, dtype=np.float32)
    assert hs.shape == (NUM_TOKENS, HIDDEN), hs.shape
    assert w.shape == (NUM_EXPERTS, HIDDEN), w.shape

    in_maps = _prep_inputs(hs, w)
    nc = _get_nc()
    try:
        if "run" not in _FN_CACHE:
            _FN_CACHE["run"] = _make_runner(nc)
        results = _FN_CACHE["run"](in_maps)
    except Exception:
        from concourse.bass_utils import run_bass_kernel_spmd

        results = run_bass_kernel_spmd(
            nc, in_maps, core_ids=list(range(N_CORES))
        ).results

    topk_idx = np.concatenate([r["topk_idx"] for r in results], axis=0)
    topk_w = np.concatenate([r["topk_w"] for r in results], axis=0)
    row_idx = (
        np.arange(NUM_TOKENS * TOP_K, dtype=np.int32).reshape(TOP_K, NUM_TOKENS).T
    )
    return (
        topk_idx.astype(np.int32),
        topk_w.astype(np.float32),
        row_idx,
    )


# revision 3
# speedup vs baseline: 1.0109x; 1.0109x over previous
"""MoE gate (router) kernel for Trainium2 — fp16 main + fp8-DoubleRow cross.

Computes, for hidden_states [T, H] and gate weight [E, H]:
    logits = hidden_states @ weight.T          # [T, E]
    probs  = softmax(logits, axis=-1)
    topk_weight, topk_idx = top_k(probs, 8)    # normalized over the top-8
    row_idx = arange(T*8).reshape(8, T).T

Strategy (8 NeuronCores, data parallel over tokens):
  - Accuracy from a 2-matmul/k-tile split (vs the 3-matmul fp16 hi/lo
    baseline): per k-tile,
      main : psum_m += hshi16 . whi16          (fp16, N=256, exact)
      cross: psum_c += hshi8.wlo8 + hslo8.whi8 (ONE fp8 DoubleRow matmul:
             DR sums lhsT[:,i,:].T @ rhs[:,i,:] over the pair dim i)
    with scale-matched fp8 operands (all in e4m3 normal range):
      hshi8 = fp8(hs)              wlo8  = fp8((w - whi16) * 2^20)
      hslo8 = fp8((hs-hshi)*2^13)  whi8s = fp8(w * 2^7)
    so both pair products carry 2^20:
      logits = psum_m + 2^-20 * psum_c
    Residual-operand errors are ~2^-18-relative -> ~2e-6 logit sigma,
    i.e. top-k exact up to a handful of 1-ulp ties (well under the
    rel-err budget).  The DR matmul covers contraction 256 in ~128
    cycles, so cross costs ~half a main matmul instead of two.
  - hs packed host-side into k-half-contiguous tiles (fp16 plane +
    pre-interleaved fp8 pair plane); each 128-token tile loads as four
    fully contiguous >=7 KiB-per-partition DMAs, keeping PE idle gaps
    under the ~3.4 us HAM re-throttle window.
  - Weights (whi16 + fp8 pair plane, ~7.3 MB) stream in chunks
    interleaved with the first token tiles; ALL weight DMAs are emitted
    before any matmul (Tile dep tracker orders only read-after-write
    for earlier writes in program order).
  - DVE max/max_index give top-8 values + indices per token in one
    instruction each; softmax over 256 experts + top-k renormalization
    reduces to a softmax over just the top-8 logits.
  - Output staging accumulates per-tile results in SBUF and leaves as
    four large-descriptor DMAs, the first three overlapping later
    tiles' compute.
"""

import numpy as np

TOP_K = 8
NUM_EXPERTS = 256
HIDDEN = 7168
NUM_TOKENS = 16384
N_CORES = 8
T_LOC = NUM_TOKENS // N_CORES

MAIN_SCALE = float(2.0**18)    # whi16 ships as fp16(w)*2^18 (exact pow2)
CROSS_SCALE = float(2.0**18)   # wlo8 carries 2^18; hslo8*whi8s = 2^12 * 2^6
HSLO_SCALE = float(2.0**12)
WHI8_SCALE = float(2.0**6)

_NC_CACHE = {}


def build_gate_nc(t_loc=T_LOC, h=HIDDEN, e=NUM_EXPERTS, repeat=1, serialize=False):
    import concourse.mybir as mybir
    import concourse.tile as tile
    from concourse import bacc
    from concourse.tile import add_dep_helper

    f32 = mybir.dt.float32
    fp16 = mybir.dt.float16
    fp8 = mybir.dt.float8e4
    P = 128
    KT = h // P          # k-tiles along hidden dim (56)
    KH = KT // 2         # k-half width (28)
    TS = t_loc // P      # 128-token subtiles per core (16)
    W_CHUNK_LENS = [2, 6, 8, 8, 8, 8, 8, 8]
    assert sum(W_CHUNK_LENS) == KT
    W_CHUNK_STARTS = [sum(W_CHUNK_LENS[:i]) for i in range(len(W_CHUNK_LENS))]

    def k_to_chunk(k):
        for ci in range(len(W_CHUNK_LENS) - 1, -1, -1):
            if k >= W_CHUNK_STARTS[ci]:
                return ci, k - W_CHUNK_STARTS[ci]
        raise AssertionError

    nc = bacc.Bacc("TRN2", target_bir_lowering=False)
    # fp16 hs plane: [p][ts][kh][t][ko2], k = p*KT + kh*KH + ko2
    hs16_pack = nc.dram_tensor(
        "hs16_pack", [P, TS, 2, P, KH], fp16, kind="ExternalInput"
    )
    # fp8 hs pair plane: [p][ts][kh][ko2][pair][t], pair 0 = hshi8, 1 = hslo8
    hs8_pack = nc.dram_tensor(
        "hs8_pack", [P, TS, 2, KH, 2, P], fp8, kind="ExternalInput"
    )
    # w fp16 plane [p][ko][e] and fp8 pair plane [p][ko][pair][e]
    w16_pack = nc.dram_tensor("w16_pack", [P, KT, e], fp16, kind="ExternalInput")
    w8_pack = nc.dram_tensor("w8_pack", [P, KT, 2, e], fp8, kind="ExternalInput")
    idx_out = nc.dram_tensor(
        "topk_idx", [t_loc, TOP_K], mybir.dt.int32, kind="ExternalOutput"
    )
    w_out = nc.dram_tensor("topk_w", [t_loc, TOP_K], f32, kind="ExternalOutput")

    with tile.TileContext(nc) as tc:
        with (
            tc.tile_pool(name="wpool", bufs=1) as wpool,
            tc.tile_pool(name="hpool", bufs=4) as hpool,
            tc.tile_pool(name="lpool", bufs=3) as lpool,
            tc.tile_pool(name="spool", bufs=4) as spool,
            tc.tile_pool(name="psum", bufs=4, space="PSUM") as psum_pool,
        ):
            if repeat > 0:
                stage_idx = wpool.tile([P, TS, TOP_K], mybir.dt.int32, tag="sidx")
                stage_wv = wpool.tile([P, TS, TOP_K], f32, tag="swv")
            w16_chunks, w8_chunks = [], []
            for wc_i, clen in enumerate(W_CHUNK_LENS):
                c16 = wpool.tile([P, clen, e], fp16, tag=f"w16_{wc_i}", name=f"w16_{wc_i}")
                c8 = wpool.tile([P, clen, 2, e], fp8, tag=f"w8_{wc_i}", name=f"w8_{wc_i}")
                w16_chunks.append(c16)
                w8_chunks.append(c8)

            def issue_w(wc_i):
                s0, l0 = W_CHUNK_STARTS[wc_i], W_CHUNK_LENS[wc_i]
                nc.sync.dma_start(w16_chunks[wc_i], w16_pack[:, s0 : s0 + l0, :])
                nc.sync.dma_start(w8_chunks[wc_i], w8_pack[:, s0 : s0 + l0, :, :])

            def issue_hs16(rep, ts_i):
                h16 = hpool.tile(
                    [P, 2, P, KH], fp16, tag="hs16", name=f"hs16_{rep}_{ts_i}"
                )
                ds = [
                    nc.sync.dma_start(h16[:, kh], hs16_pack[:, ts_i, kh])
                    for kh in range(2)
                ]
                return h16, ds

            def issue_hs8(rep, ts_i):
                h8 = hpool.tile(
                    [P, 2, KH, 2, P], fp8, tag="hs8", name=f"hs8_{rep}_{ts_i}"
                )
                ds = [
                    nc.sync.dma_start(h8[:, kh], hs8_pack[:, ts_i, kh])
                    for kh in range(2)
                ]
                return h8, ds

            def issue_hs(rep, ts_i):
                # four DMAs per 128-token tile (fp16/fp8 x k-halves), each
                # fully contiguous >=7,168 B per partition
                h16, d16 = issue_hs16(rep, ts_i)
                h8, d8 = issue_hs8(rep, ts_i)
                return h16, h8, d16 + d8

            N_PRE = min(4, TS) if repeat > 0 else 0
            pre_hs = {}
            dma_done = {}
            _clock = [0.0]

            def _track(kind, key, nbytes):
                _clock[0] += nbytes / 360.0
                dma_done[(kind, key)] = _clock[0] + 1500.0

            H16_BYTES = P * P * KH * 2
            H8_BYTES = P * KH * 2 * P

            def w_bytes(ci):
                return P * W_CHUNK_LENS[ci] * e * (2 + 2)

            def track_hs(ts_i):
                _track("h16_0", ts_i, H16_BYTES)
                _track("h8_0", ts_i, H8_BYTES)
                _track("h16_1", ts_i, H16_BYTES)
                _track("h8_1", ts_i, H8_BYTES)

            issue_w(0); _track("w", 0, w_bytes(0))
            if N_PRE > 0:
                pre_hs[0] = issue_hs(0, 0); track_hs(0)
            issue_w(1); _track("w", 1, w_bytes(1))
            if N_PRE > 1:
                pre_hs[1] = issue_hs(0, 1); track_hs(1)
            issue_w(2); _track("w", 2, w_bytes(2))
            issue_w(3); _track("w", 3, w_bytes(3))
            if N_PRE > 2:
                pre_hs[2] = issue_hs(0, 2); track_hs(2)
            issue_w(4); _track("w", 4, w_bytes(4))
            issue_w(5); _track("w", 5, w_bytes(5))
            if N_PRE > 3:
                pre_hs[3] = issue_hs(0, 3); track_hs(3)
            issue_w(6); _track("w", 6, w_bytes(6))
            issue_w(7); _track("w", 7, w_bytes(7))

            def mm_main(ptm, h16, k, start, stop):
                ci, ki = k_to_chunk(k)
                nc.tensor.matmul(
                    ptm, h16[:, k // KH, :, k % KH], w16_chunks[ci][:, ki, :],
                    start=start, stop=stop,
                )

            def mm_cross(ptm, h8, k, start, stop):
                ci, ki = k_to_chunk(k)
                nc.tensor.matmul(
                    ptm, h8[:, k // KH, k % KH], w8_chunks[ci][:, ki],
                    start=start, stop=stop,
                    perf_mode=mybir.MatmulPerfMode.DoubleRow,
                )

            # fill-phase matmuls for prefetched tiles, in data-ready order
            pre_pt = {}
            if N_PRE:
                sched = []
                for ts_i in range(N_PRE):
                    ptm = psum_pool.tile([P, e], f32, tag="ptm", name=f"ptm_pre{ts_i}")
                    pre_pt[ts_i] = ptm
                    for k in range(KT):
                        ci, _ = k_to_chunk(k)
                        kh = k // KH
                        rdy_m = max(dma_done[("w", ci)], dma_done[(f"h16_{kh}", ts_i)])
                        rdy_c = max(dma_done[("w", ci)], dma_done[(f"h8_{kh}", ts_i)])
                        # kind-major within each readiness step keeps same-mode
                        # matmuls grouped (mode switches stall the PE weight
                        # pipeline)
                        sched.append((rdy_m, 0, ts_i, k))
                        sched.append((rdy_c, 1, ts_i, k))
                sched.sort()
                started = set()
                last_of = {}
                for i, (_, kind, ts_i, k) in enumerate(sched):
                    last_of[ts_i] = i
                for i, (_, kind, ts_i, k) in enumerate(sched):
                    h16, h8, _ = pre_hs[ts_i]
                    ptm = pre_pt[ts_i]
                    st = ts_i not in started
                    started.add(ts_i)
                    sp = i == last_of[ts_i]
                    if kind == 0:
                        mm_main(ptm, h16, k, st, sp)
                    else:
                        mm_cross(ptm, h8, k, st, sp)

            def epilogue(rep, ts_i, ptm):
                nonlocal prev_tail
                # psum holds 2^18 * logits (main and cross share the scale);
                # top-8 straight off the PSUM, descale folds into Exp's scale
                mx = spool.tile([P, TOP_K], f32, tag="mx")
                nc.vector.max(out=mx, in_=ptm)
                idx_u = spool.tile([P, TOP_K], mybir.dt.uint32, tag="idxu")
                nc.vector.max_index(idx_u, mx, ptm)
                nc.vector.tensor_copy(stage_idx[:, ts_i, :], idx_u)
                nm = spool.tile([P, 1], f32, tag="nm")
                nc.vector.tensor_scalar_mul(nm, mx[:, 0:1], -1.0 / MAIN_SCALE)
                ev = spool.tile([P, TOP_K], f32, tag="ev")
                sm = spool.tile([P, 1], f32, tag="sm")
                nc.scalar.activation(
                    ev,
                    mx,
                    mybir.ActivationFunctionType.Exp,
                    bias=nm,
                    scale=1.0 / MAIN_SCALE,
                    accum_out=sm,
                )
                rc = spool.tile([P, 1], f32, tag="rc")
                nc.vector.reciprocal(rc, sm)
                tail = nc.vector.tensor_scalar_mul(stage_wv[:, ts_i, :], ev, rc)
                if serialize and ts_i == TS - 1:
                    prev_tail = tail
                if rep == repeat - 1 and (ts_i + 1) % (TS // 4) == 0 and ts_i != TS - 1:
                    q0 = (ts_i + 1 - TS // 4) * P
                    q1 = (ts_i + 1) * P
                    nc.sync.dma_start(
                        idx_out[q0:q1, :].rearrange("(ts p) k -> p ts k", p=P),
                        stage_idx[:, ts_i + 1 - TS // 4 : ts_i + 1, :],
                    )
                    nc.sync.dma_start(
                        w_out[q0:q1, :].rearrange("(ts p) k -> p ts k", p=P),
                        stage_wv[:, ts_i + 1 - TS // 4 : ts_i + 1, :],
                    )

            prev_tail = None
            for rep in range(repeat):
                ts_i = 0
                while ts_i < TS:
                    if rep == 0 and ts_i < N_PRE:
                        h16, h8, ds = pre_hs[ts_i]
                        epilogue(rep, ts_i, pre_pt[ts_i])
                        ts_i += 1
                        continue
                    # 2-tile groups: A(t0) A(t1) B(t0) B(t1) halves the
                    # fp16<->DoubleRow mode transitions (each stalls the PE
                    # weight pipeline ~0.2 us); h16 DMAs lead h8 so the A
                    # phases are fed first
                    group = [ts_i] + ([ts_i + 1] if ts_i + 1 < TS else [])
                    datas = []
                    for t in group:
                        h16, d16 = issue_hs16(rep, t)
                        datas.append([t, h16, None, None, None, d16])
                    for di, t in enumerate(group):
                        h8, d8 = issue_hs8(rep, t)
                        datas[di][2] = h8
                        datas[di][5] = datas[di][5] + d8
                    if serialize and prev_tail is not None:
                        for row in datas:
                            for d in row[5]:
                                add_dep_helper(prev_tail.ins, d.ins, reason="ser-rep")
                    for row in datas:
                        row[3] = psum_pool.tile(
                            [P, e], f32, tag="ptm", name=f"ptm{rep}_{row[0]}"
                        )
                    for t, h16, h8, ptm, ptc, _ in datas:
                        for k in range(KT):
                            mm_main(ptm, h16, k, k == 0, False)
                    for t, h16, h8, ptm, ptc, _ in datas:
                        for k in range(KT):
                            mm_cross(ptm, h8, k, False, k == KT - 1)
                    for t, h16, h8, ptm, ptc, _ in datas:
                        epilogue(rep, t, ptm)
                    ts_i += len(group)
            if repeat > 0:
                q0 = (TS - TS // 4) * P
                nc.sync.dma_start(
                    idx_out[q0:, :].rearrange("(ts p) k -> p ts k", p=P),
                    stage_idx[:, TS - TS // 4 :, :],
                )
                nc.sync.dma_start(
                    w_out[q0:, :].rearrange("(ts p) k -> p ts k", p=P),
                    stage_wv[:, TS - TS // 4 :, :],
                )
    nc.compile()
    return nc


def _get_nc():
    key = (T_LOC, HIDDEN, NUM_EXPERTS)
    if key not in _NC_CACHE:
        _NC_CACHE[key] = build_gate_nc(*key)
    return _NC_CACHE[key]


def _prep_inputs(hs, w):
    from concourse import mybir

    f8 = mybir.dt.np(mybir.dt.float8e4)
    P = 128
    KT = HIDDEN // P
    KH = KT // 2
    TS = T_LOC // P
    E = NUM_EXPERTS

    whi = w.astype(np.float16)
    wres = (w.astype(np.float64) - whi.astype(np.float64)).astype(np.float32)
    wlo8 = (wres * np.float32(CROSS_SCALE)).astype(f8)
    whi8s = (w * np.float32(WHI8_SCALE)).astype(f8)
    # main plane pre-scaled by 2^18 (exact power-of-2 in fp16) so main and
    # cross accumulate into ONE psum region at matching scale
    whi_s = (whi.astype(np.float32) * np.float32(MAIN_SCALE)).astype(np.float16)
    assert np.isfinite(whi_s.astype(np.float32)).all()
    # [E, H] -> [H, E] -> [p, ko, e]
    w16_pack = np.ascontiguousarray(whi_s.T).reshape(P, KT, E)
    w8_pack = np.empty((P, KT, 2, E), dtype=f8)
    w8_pack[:, :, 0, :] = np.ascontiguousarray(wlo8.T).reshape(P, KT, E)
    w8_pack[:, :, 1, :] = np.ascontiguousarray(whi8s.T).reshape(P, KT, E)

    def pack_core(c):
        hs_c = hs[c * T_LOC : (c + 1) * T_LOC]  # [T_LOC, H] f32
        hshi = hs_c.astype(np.float16)
        hsres = (hs_c.astype(np.float64) - hshi.astype(np.float64)).astype(
            np.float32
        )
        hshi8 = hs_c.astype(f8)
        hslo8 = (hsres * np.float32(HSLO_SCALE)).astype(f8)
        # fp16 plane: [ts, t, p, kh, ko2] -> [p, ts, kh, t, ko2]
        p16 = np.ascontiguousarray(
            hshi.reshape(TS, P, P, 2, KH).transpose(2, 0, 3, 1, 4)
        )
        # fp8 pair plane: [ts, t, p, kh, ko2] -> [p, ts, kh, ko2, pair, t]
        p8 = np.empty((P, TS, 2, KH, 2, P), dtype=f8)
        p8[:, :, :, :, 0, :] = hshi8.reshape(TS, P, P, 2, KH).transpose(
            2, 0, 3, 4, 1
        )
        p8[:, :, :, :, 1, :] = hslo8.reshape(TS, P, P, 2, KH).transpose(
            2, 0, 3, 4, 1
        )
        return {
            "hs16_pack": p16,
            "hs8_pack": p8,
            "w16_pack": w16_pack,
            "w8_pack": w8_pack,
        }

    from concurrent.futures import ThreadPoolExecutor

    with ThreadPoolExecutor(max_workers=N_CORES) as ex:
        in_maps = list(ex.map(pack_core, range(N_CORES)))
    return in_maps


_FN_CACHE = {}


def _make_runner(nc):
    """Compile a reusable 8-core PJRT callable."""
    import jax
    import concourse.mybir as mybir
    from concourse import bass2jax
    from jax.sharding import Mesh, NamedSharding, PartitionSpec
    from jax.experimental.shard_map import shard_map

    bass2jax.install_neuronx_cc_hook()
    partition_name = nc.partition_id_tensor.name if nc.partition_id_tensor else None
    in_names, out_names, out_avals, zero_shapes = [], [], [], []
    for alloc in nc.m.functions[0].allocations:
        if not isinstance(alloc, mybir.MemoryLocationSet):
            continue
        name = alloc.memorylocations[0].name
        if alloc.kind == "ExternalInput":
            if name != partition_name:
                in_names.append(name)
        elif alloc.kind == "ExternalOutput":
            shape = tuple(alloc.tensor_shape)
            dtype = mybir.dt.np(alloc.dtype)
            out_names.append(name)
            out_avals.append(jax.core.ShapedArray(shape, dtype))
            zero_shapes.append((shape, dtype))
    n_params = len(in_names)
    n_outs = len(out_avals)
    all_in_names = list(in_names) + list(out_names)
    if partition_name is not None:
        all_in_names.append(partition_name)

    def _body(*args):
        operands = list(args)
        if partition_name is not None:
            operands.append(bass2jax.partition_id_tensor())
        outs = bass2jax._bass_exec_p.bind(
            *operands,
            out_avals=tuple(out_avals),
            in_names=tuple(all_in_names),
            out_names=tuple(out_names),
            lowering_input_output_aliases=(),
            sim_require_finite=True,
            sim_require_nnan=True,
            nc=nc,
        )
        return tuple(outs)

    devices = jax.devices()[:N_CORES]
    mesh = Mesh(np.asarray(devices), ("core",))
    in_specs = (PartitionSpec("core"),) * (n_params + n_outs)
    out_specs = (PartitionSpec("core"),) * len(out_names)
    donate = tuple(range(n_params, n_params + n_outs))
    fn = jax.jit(
        shard_map(
            _body, mesh=mesh, in_specs=in_specs, out_specs=out_specs, check_rep=False
        ),
        donate_argnums=donate,
        keep_unused=True,
    )
    sharding = NamedSharding(mesh, PartitionSpec("core"))

    def run(in_maps):
        concat_in = [
            np.concatenate(
                [np.asarray(in_maps[c][nm]) for c in range(N_CORES)], axis=0
            )
            for nm in in_names
        ]
        zeros = [
            np.zeros((N_CORES * s[0], *s[1:]), dt) for s, dt in zero_shapes
        ]
        dev_in = [jax.device_put(x, sharding) for x in concat_in]
        out_arrs = fn(*dev_in, *zeros)
        return [
            {
                nm: np.asarray(out_arrs[i]).reshape(
                    N_CORES, *out_avals[i].shape
                )[c]
                for i, nm in enumerate(out_names)
            }
            for c in range(N_CORES)
        ]

    return run


def kernel(hidden_states, weight):
    hs = np.asarray(hidden_states, dtype=np.float32)
    w = np.asarray(weight, dtype=np.float32)
    assert hs.shape == (NUM_TOKENS, HIDDEN), hs.shape
    assert w.shape == (NUM_EXPERTS, HIDDEN), w.shape

    in_maps = _prep_inputs(hs, w)
    nc = _get_nc()
    try:
        if "run" not in _FN_CACHE:
            _FN_CACHE["run"] = _make_runner(nc)
        results = _FN_CACHE["run"](in_maps)
    except Exception:
        from concourse.bass_utils import run_bass_kernel_spmd

        results = run_bass_kernel_spmd(
            nc, in_maps, core_ids=list(range(N_CORES))
        ).results

    topk_idx = np.concatenate([r["topk_idx"] for r in results], axis=0)
    topk_w = np.concatenate([r["topk_w"] for r in results], axis=0)
    row_idx = (
        np.arange(NUM_TOKENS * TOP_K, dtype=np.int32).reshape(TOP_K, NUM_TOKENS).T
    )
    return (
        topk_idx.astype(np.int32),
        topk_w.astype(np.float32),
        row_idx,
    )
